# revision 11
# baseline (speedup 1.0000x reference)
"""Sparse attention (B=4,H=16,N=2048,D=64) on 8 trn2 NeuronCores.

Sharding: core c = bp*4 + hq handles batches [2bp, 2bp+1] x heads [4hq..4hq+3].
Per (b,h), with host-precomputed mex = mask^T * exp(bias^T) / 16 streamed in:
  P^T = exp(K Q^T/8) * mex          (exp on ACT, mul on DVE)
  [O~ ; denom]^T = [V | 1]^T @ P^T  (accumulated fp32 in PSUM)
Device ships unnormalized [O~; denom] as fp16; host divides + transposes.

Flat software-pipelined loop over 128 key-tile-pair iterations per core:
S-matmuls for iter t run 2 iterations ahead of the O-matmuls (lag-2) so
the ACT engine (exp, the pacer at ~1.07us/tile) never starves behind
O-matmuls that wait on the DVE multiply.  Adjacent key tiles' S-matmuls
run concurrently in PE row groups 0-63/64-127 (q replicated in both
partition halves; k parity-packed so odd tiles live in rows 64-127).
"""

import numpy as np
import ml_dtypes

import concourse.bass as bass
from concourse import bacc
import concourse.mybir as mybir
import concourse.tile as tile
from concourse.bass_utils import run_bass_kernel_spmd

dt = mybir.dt
AF = mybir.ActivationFunctionType

B, H, N, D = 4, 16, 2048, 64
NB = 2    # batches per core
NH = 4    # heads per core
P = 128
NKT = N // P          # 16 key tiles
NKT2 = NKT // 2       # 8 key-tile pairs
QW = 512              # matmul free-dim (one PSUM bank of fp32)
SUPW = 1024           # S tile width / ACT width (2 PSUM banks)
NQP = N // SUPW       # 2 query supertiles
NPAIR = NB * NH       # 8 (b,h) pairs per core
T_ITER = NPAIR * NQP * NKT2   # 128 pipeline iterations
LAG = 2
SCALE = np.float32(1.0 / 16.0)   # folded into mex; cancels in normalization
TRACE = False

_CACHE = {}


def build_bass():
    nc = bacc.Bacc()
    # q^T scaled, replicated into both partition halves: [NB,NH,128,N]
    qT = nc.declare_dram_parameter("qT", [NB, NH, 2 * D, N], dt.float16, isOutput=False)
    # k^T parity-packed: rows 0-63 even key tiles, rows 64-127 odd: [NB,NH,128,N/2]
    kT = nc.declare_dram_parameter("kT", [NB, NH, 2 * D, N // 2], dt.float16, isOutput=False)
    # [V | 1] pre-tiled: [NB,NH,128,NKT*(D+1)]
    vA = nc.declare_dram_parameter("vA", [NB, NH, P, NKT * (D + 1)], dt.float16, isOutput=False)
    # mask*exp(bias)*SCALE, tiled per iteration: [T_ITER, 128, 2*SUPW]
    mex = nc.declare_dram_parameter("mex", [T_ITER, P, 2 * SUPW], dt.float16, isOutput=False)
    # unnormalized [O~; denom]^T per pair: [NB,NH,D+1,N]
    outU = nc.declare_dram_parameter("outU", [NB, NH, D + 1, N], dt.float16, isOutput=True)

    def sched(t):
        pair, r = divmod(t, NQP * NKT2)
        qp, ktp = divmod(r, NKT2)
        h, b = divmod(pair, NB)
        return pair, b, h, qp, ktp

    with tile.TileContext(nc) as tc:
        with (
            tc.tile_pool(name="qk", bufs=2) as qkpool,
            tc.tile_pool(name="vp", bufs=2) as vpool,
            tc.tile_pool(name="mex", bufs=6) as mpool,
            tc.tile_pool(name="pt", bufs=6) as ppool,
            tc.tile_pool(name="out", bufs=2) as opool_sb,
            tc.tile_pool(name="spsum", bufs=1, space="PSUM") as spool,
            tc.tile_pool(name="opsum", bufs=1, space="PSUM") as opool,
        ):
            qkv = [None] * NPAIR   # (qsb, ksb, vsb) per pair
            pts = [None] * T_ITER  # pt tile + indices for the O-side
            osum = [None]          # current opsum tiles

            # persistent 6-bank S buffer; iteration t writes its two key
            # tiles into 1024-wide slots (2t)%3 and (2t+1)%3.  When the two
            # slots are adjacent (2/3 of iterations) a single 2048-wide
            # ACTIVATE computes both exps, saving per-instruction overhead.
            sbig = spool.tile([P, 3 * SUPW], dt.float32, tag="sbig")

            def load_pair(p):
                h, b = divmod(p, NB)
                qt_ = qkpool.tile([2 * D, N], dt.float16, tag="q")
                kt_ = qkpool.tile([2 * D, N // 2], dt.float16, tag="k")
                vt_ = vpool.tile([P, NKT * (D + 1)], dt.float16, tag="v")
                if p == 0:
                    # ramp: spread the critical q/k loads over idle engine
                    # queues so they aren't starved by the mex prefetch burst.
                    # q goes at the HEAD of the sync queue (before all mex),
                    # k alone on the scalar queue (safe: no ACTIVATE yet).
                    nc.sync.dma_start(qt_, qT[b, h])
                    nc.scalar.dma_start(kt_, kT[b, h])
                    nc.gpsimd.dma_start(vt_, vA[b, h])
                else:
                    nc.gpsimd.dma_start(qt_, qT[b, h])
                    nc.gpsimd.dma_start(kt_, kT[b, h])
                    nc.gpsimd.dma_start(vt_, vA[b, h])
                qkv[p] = (qt_, kt_, vt_)

            load_pair(0)
            for t in range(T_ITER + LAG):
                # ---------------- S side: S-matmuls, exp, mex multiply ----
                if t < T_ITER:
                    pair, b, h, qp, ktp = sched(t)
                    q0 = qp * SUPW
                    # prefetch next pair's q/k/v mid-way through this pair
                    if t % (NQP * NKT2) == NKT2 and pair + 1 < NPAIR:
                        load_pair(pair + 1)
                    qsb, ksb, vsb = qkv[pair]

                    mexsb = mpool.tile([P, 2 * SUPW], dt.float16, tag="mex")
                    nc.sync.dma_start(mexsb, mex[t])

                    pt = ppool.tile([P, 2 * SUPW], dt.float16, tag="pt")
                    slots = [(2 * t) % 3, (2 * t + 1) % 3]
                    # adjacent key tiles (j=0,1) -> PE row groups 0/64
                    ti = ktp  # column block in parity-packed k
                    for j in range(2):
                        s0 = slots[j] * SUPW
                        for qi in range(2):
                            rg = j * D
                            nc.tensor.matmul(
                                sbig[:, s0 + qi * QW:s0 + (qi + 1) * QW],
                                ksb[rg:rg + D, ti * P:(ti + 1) * P],
                                qsb[rg:rg + D,
                                    q0 + qi * QW:q0 + (qi + 1) * QW],
                                start=True, stop=True,
                                tile_position=(rg, 0),
                            )
                    if slots[1] == slots[0] + 1:
                        nc.scalar.activation(
                            pt, sbig[:, slots[0] * SUPW:
                                     (slots[0] + 2) * SUPW], AF.Exp)
                    else:
                        for j in range(2):
                            nc.scalar.activation(
                                pt[:, j * SUPW:(j + 1) * SUPW],
                                sbig[:, slots[j] * SUPW:
                                     (slots[j] + 1) * SUPW], AF.Exp)
                    nc.vector.tensor_mul(pt, pt, mexsb)
                    pts[t] = (pt, pair, b, h, qp, ktp)

                # ---------------- O side (lag-2): accumulate V~ @ P^T -----
                to = t - LAG
                if to >= 0:
                    pt, pair, b, h, qp, ktp = pts[to]
                    pts[to] = None
                    vsb = qkv[pair][2]
                    if ktp == 0:
                        osum[0] = [
                            opool.tile([D + 1, QW], dt.float32,
                                       tag=f"o{qi}", name=f"opsum{qi}")
                            for qi in range(2)
                        ]
                    opsum = osum[0]
                    for j in range(2):
                        kt = 2 * ktp + j
                        for qi in range(2):
                            nc.tensor.matmul(
                                opsum[qi],
                                vsb[:, kt * (D + 1):(kt + 1) * (D + 1)],
                                pt[:, j * SUPW + qi * QW:
                                   j * SUPW + (qi + 1) * QW],
                                start=(kt == 0), stop=(kt == NKT - 1),
                            )
                    if ktp == NKT2 - 1:
                        # drain unnormalized accumulators to SBUF as fp16
                        q0 = qp * SUPW
                        osb = opool_sb.tile([D + 1, SUPW], dt.float16,
                                            tag="osb")
                        for qi in range(2):
                            nc.vector.tensor_copy(
                                osb[:, qi * QW:(qi + 1) * QW], opsum[qi])
                        nc.gpsimd.dma_start(
                            outU[b, h, :, q0:q0 + SUPW], osb)
    nc.finalize()
    return nc


def make_in_maps(q, k, v, mask, attn_bias):
    scale = np.float32(D ** -0.5)
    qTf = (q.transpose(0, 1, 3, 2) * scale).astype(np.float16)   # [B,H,D,N]
    kTf = k.transpose(0, 1, 3, 2).astype(np.float16)             # [B,H,D,N]
    # parity-packed k^T: [B,H,2,D,N/2] -> [B,H,2D,N/2]
    kPar = np.ascontiguousarray(
        kTf.reshape(B, H, D, NKT2, 2, P).transpose(0, 1, 4, 2, 3, 5)
        .reshape(B, H, 2 * D, N // 2))
    vA = np.concatenate(
        [v, np.ones((B, H, N, 1), np.float32)], axis=-1
    ).astype(np.float16)                                         # [B,H,N,D+1]
    # pre-tiled: [B,H,P,NKT*(D+1)]
    vTile = np.ascontiguousarray(
        vA.reshape(B, H, NKT, P, D + 1).transpose(0, 1, 3, 2, 4)
        .reshape(B, H, P, NKT * (D + 1)))
    # transposed [key, query] views
    maskT = mask[:, 0].transpose(0, 2, 1)                        # [B,N,N] bool
    expbT = (np.exp(attn_bias[0].transpose(0, 2, 1),
                    dtype=np.float32) * SCALE).astype(np.float16)  # [H,N,N]

    in_maps = []
    for c in range(8):
        bp, hq = divmod(c, 4)
        bs, hs = 2 * bp, 4 * hq
        qrep = np.concatenate([qTf[bs:bs + NB, hs:hs + NH]] * 2, axis=2)
        # mex[t] = [128, 2*SUPW] for iteration t = ((h*NB+b)*NQP+qp)*NKT2+ktp
        # mex[t][p, j*SUPW+u] = maskT[key=(2ktp+j)*128+p, query=qp*SUPW+u]
        #                       * expbT[h][key, query] * SCALE
        mex = np.empty((T_ITER, P, 2 * SUPW), np.float16)
        for h in range(NH):
            for b in range(NB):
                m = np.where(maskT[bs + b], expbT[hs + h], np.float16(0))
                # [NKT2, 2, P, NQP, SUPW] -> [NQP, NKT2, P, 2, SUPW]
                mt = (m.reshape(NKT2, 2, P, NQP, SUPW)
                      .transpose(3, 0, 2, 1, 4)
                      .reshape(NQP, NKT2, P, 2 * SUPW))
                pairi = h * NB + b
                t0 = pairi * NQP * NKT2
                mex[t0:t0 + NQP * NKT2] = mt.reshape(
                    NQP * NKT2, P, 2 * SUPW)
        in_maps.append({
            "qT": np.ascontiguousarray(qrep),
            "kT": np.ascontiguousarray(kPar[bs:bs + NB, hs:hs + NH]),
            "vA": np.ascontiguousarray(vTile[bs:bs + NB, hs:hs + NH]),
            "mex": mex,
        })
    return in_maps


def kernel(q, k, v, mask, attn_bias):
    if "nc" not in _CACHE:
        _CACHE["nc"] = build_bass()
    nc = _CACHE["nc"]
    in_maps = make_in_maps(
        np.asarray(q, np.float32), np.asarray(k, np.float32),
        np.asarray(v, np.float32), np.asarray(mask, bool),
        np.asarray(attn_bias, np.float32),
    )
    rr = run_bass_kernel_spmd(
        nc, in_maps, list(range(8)), trace=TRACE,
        tmpdir=_CACHE.get("tmpdir"),
    )
    _CACHE["last_result"] = rr

    out = np.empty((B, H, N, D), np.float32)
    for c in range(8):
        bp, hq = divmod(c, 4)
        bs, hs = 2 * bp, 4 * hq
        oU = np.asarray(rr.results[c]["outU"]).astype(np.float32)  # [NB,NH,65,N]
        o = oU[:, :, :D, :] / oU[:, :, D:D + 1, :]
        out[bs:bs + NB, hs:hs + NH] = o.transpose(0, 1, 3, 2)
    return out


# revision 17
# speedup vs baseline: 1.3589x; 1.3589x over previous
"""Sparse attention (B=4,H=16,N=2048,D=64) on 8 trn2 NeuronCores.

Sharding: core c = bp*4 + hq handles batches [2bp, 2bp+1] x heads [4hq..4hq+3].
Per (b,h), with host-precomputed mex = mask^T * exp(bias^T) / 16 streamed in:
  P^T = exp(K Q^T/8) * mex          (exp on ACT, mul on DVE)
  [O~ ; denom]^T = [V | 1]^T @ P^T  (accumulated fp32 in PSUM)
Device ships unnormalized [O~; denom] as fp16; host divides + transposes.

Flat software-pipelined loop over 128 key-tile-pair iterations per core:
S-matmuls for iter t run 2 iterations ahead of the O-matmuls (lag-2) so
the ACT engine (exp, the pacer at ~1.07us/tile) never starves behind
O-matmuls that wait on the DVE multiply.  Adjacent key tiles' S-matmuls
run concurrently in PE row groups 0-63/64-127 (q replicated in both
partition halves; k parity-packed so odd tiles live in rows 64-127).
"""

import numpy as np
import ml_dtypes

import concourse.bass as bass
from concourse import bacc
import concourse.mybir as mybir
import concourse.tile as tile
from concourse.bass_utils import run_bass_kernel_spmd

dt = mybir.dt
AF = mybir.ActivationFunctionType

B, H, N, D = 4, 16, 2048, 64
NB = 2    # batches per core
NH = 4    # heads per core
P = 128
NKT = N // P          # 16 key tiles
NKT2 = NKT // 2       # 8 key-tile pairs
QW = 512              # matmul free-dim (one PSUM bank of fp32)
SUPW = 1024           # S tile width / ACT width (2 PSUM banks)
NQP = N // SUPW       # 2 query supertiles
NPAIR = NB * NH       # 8 (b,h) pairs per core
T_ITER = NPAIR * NQP * NKT2   # 128 pipeline iterations
LAG = 2
SCALE = np.float32(1.0 / 16.0)   # folded into mex; cancels in normalization
TRACE = False

_CACHE = {}


def build_bass():
    nc = bacc.Bacc()
    # q^T scaled, replicated into both partition halves: [NB,NH,128,N]
    qT = nc.declare_dram_parameter("qT", [NB, NH, 2 * D, N], dt.float16, isOutput=False)
    # k^T zero-interleaved: even key tiles in rows 0-63, odd in rows 64-127,
    # zeros elsewhere, so S-matmuls contract over 128 partitions (full mode)
    kT = nc.declare_dram_parameter("kT", [NB, NH, 2 * D, N], dt.float16, isOutput=False)
    # [V | 1] pre-tiled: [NB,NH,128,NKT*(D+1)]
    vA = nc.declare_dram_parameter("vA", [NB, NH, P, NKT * (D + 1)], dt.float16, isOutput=False)
    # mask*exp(bias)*SCALE, tiled per iteration: [T_ITER, 128, 2*SUPW]
    mex = nc.declare_dram_parameter("mex", [T_ITER, P, 2 * SUPW], dt.float16, isOutput=False)
    # unnormalized [O~; denom]^T per pair: [NB,NH,D+1,N]
    outU = nc.declare_dram_parameter("outU", [NB, NH, D + 1, N], dt.float16, isOutput=True)

    def sched(t):
        pair, r = divmod(t, NQP * NKT2)
        qp, ktp = divmod(r, NKT2)
        h, b = divmod(pair, NB)
        return pair, b, h, qp, ktp

    with tile.TileContext(nc) as tc:
        with (
            tc.tile_pool(name="qk", bufs=2) as qkpool,
            tc.tile_pool(name="vp", bufs=2) as vpool,
            tc.tile_pool(name="mex", bufs=6) as mpool,
            tc.tile_pool(name="pt", bufs=6) as ppool,
            tc.tile_pool(name="out", bufs=2) as opool_sb,
            tc.tile_pool(name="spsum", bufs=3, space="PSUM") as spool,
            tc.tile_pool(name="opsum", bufs=1, space="PSUM") as opool,
        ):
            qkv = [None] * NPAIR   # (qsb, ksb, vsb) per pair
            pts = [None] * T_ITER  # pt tile + indices for the O-side
            osum = [None]          # current opsum tiles

            def load_pair(p):
                h, b = divmod(p, NB)
                qt_ = qkpool.tile([2 * D, N], dt.float16, tag="q")
                kt_ = qkpool.tile([2 * D, N], dt.float16, tag="k")
                vt_ = vpool.tile([P, NKT * (D + 1)], dt.float16, tag="v")
                if p == 0:
                    # ramp: spread the critical q/k loads over idle engine
                    # queues so they aren't starved by the mex prefetch burst.
                    # q goes at the HEAD of the sync queue (before all mex),
                    # k alone on the scalar queue (safe: no ACTIVATE yet).
                    nc.sync.dma_start(qt_, qT[b, h])
                    nc.scalar.dma_start(kt_, kT[b, h])
                    nc.gpsimd.dma_start(vt_, vA[b, h])
                else:
                    nc.gpsimd.dma_start(qt_, qT[b, h])
                    nc.gpsimd.dma_start(kt_, kT[b, h])
                    nc.gpsimd.dma_start(vt_, vA[b, h])
                qkv[p] = (qt_, kt_, vt_)

            load_pair(0)
            for t in range(T_ITER + LAG):
                # ---------------- S side: S-matmuls, exp, mex multiply ----
                if t < T_ITER:
                    pair, b, h, qp, ktp = sched(t)
                    q0 = qp * SUPW
                    # prefetch next pair's q/k/v mid-way through this pair
                    if t % (NQP * NKT2) == NKT2 and pair + 1 < NPAIR:
                        load_pair(pair + 1)
                    qsb, ksb, vsb = qkv[pair]

                    mexsb = mpool.tile([P, 2 * SUPW], dt.float16, tag="mex")
                    nc.sync.dma_start(mexsb, mex[t])

                    pt = ppool.tile([P, 2 * SUPW], dt.float16, tag="pt")
                    ssups = []
                    for j in range(2):
                        ss = spool.tile([P, SUPW], dt.float32, tag="s",
                                        name=f"ssup{j}")
                        ssups.append(ss)
                    # k is zero-interleaved on host (even key tiles live in
                    # rows 0-63, odd in 64-127, zeros elsewhere) so every
                    # matmul runs in full (128,128) mode - no PE tiling-mode
                    # switches between the S and O matmul groups.
                    for j in range(2):
                        kt = 2 * ktp + j
                        for qi in range(2):
                            nc.tensor.matmul(
                                ssups[j][:, qi * QW:(qi + 1) * QW],
                                ksb[:, kt * P:(kt + 1) * P],
                                qsb[:, q0 + qi * QW:q0 + (qi + 1) * QW],
                                start=True, stop=True,
                            )
                    for j in range(2):
                        nc.scalar.activation(
                            pt[:, j * SUPW:(j + 1) * SUPW], ssups[j], AF.Exp)
                    nc.vector.tensor_mul(pt, pt, mexsb)
                    pts[t] = (pt, pair, b, h, qp, ktp)

                # ---------------- O side (lag-2): accumulate V~ @ P^T -----
                to = t - LAG
                if to >= 0:
                    pt, pair, b, h, qp, ktp = pts[to]
                    pts[to] = None
                    vsb = qkv[pair][2]
                    if ktp == 0:
                        osum[0] = [
                            opool.tile([D + 1, QW], dt.float32,
                                       tag=f"o{qi}", name=f"opsum{qi}")
                            for qi in range(2)
                        ]
                    opsum = osum[0]
                    for j in range(2):
                        kt = 2 * ktp + j
                        for qi in range(2):
                            nc.tensor.matmul(
                                opsum[qi],
                                vsb[:, kt * (D + 1):(kt + 1) * (D + 1)],
                                pt[:, j * SUPW + qi * QW:
                                   j * SUPW + (qi + 1) * QW],
                                start=(kt == 0), stop=(kt == NKT - 1),
                            )
                    if ktp == NKT2 - 1:
                        # drain unnormalized accumulators to SBUF as fp16
                        q0 = qp * SUPW
                        osb = opool_sb.tile([D + 1, SUPW], dt.float16,
                                            tag="osb")
                        for qi in range(2):
                            nc.vector.tensor_copy(
                                osb[:, qi * QW:(qi + 1) * QW], opsum[qi])
                        nc.gpsimd.dma_start(
                            outU[b, h, :, q0:q0 + SUPW], osb)
    nc.finalize()
    return nc


def make_in_maps(q, k, v, mask, attn_bias):
    scale = np.float32(D ** -0.5)
    qTf = (q.transpose(0, 1, 3, 2) * scale).astype(np.float16)   # [B,H,D,N]
    kTf = k.transpose(0, 1, 3, 2).astype(np.float16)             # [B,H,D,N]
    # zero-interleaved k^T: even key tiles in rows 0-63, odd in 64-127
    kPar = np.zeros((B, H, 2 * D, NKT, P), np.float16)
    kv = kTf.reshape(B, H, D, NKT, P)
    kPar[:, :, :D, 0::2] = kv[:, :, :, 0::2]
    kPar[:, :, D:, 1::2] = kv[:, :, :, 1::2]
    kPar = np.ascontiguousarray(kPar.reshape(B, H, 2 * D, N))
    vA = np.concatenate(
        [v, np.ones((B, H, N, 1), np.float32)], axis=-1
    ).astype(np.float16)                                         # [B,H,N,D+1]
    # pre-tiled: [B,H,P,NKT*(D+1)]
    vTile = np.ascontiguousarray(
        vA.reshape(B, H, NKT, P, D + 1).transpose(0, 1, 3, 2, 4)
        .reshape(B, H, P, NKT * (D + 1)))
    # transposed [key, query] views
    maskT = mask[:, 0].transpose(0, 2, 1)                        # [B,N,N] bool
    expbT = (np.exp(attn_bias[0].transpose(0, 2, 1),
                    dtype=np.float32) * SCALE).astype(np.float16)  # [H,N,N]

    in_maps = []
    for c in range(8):
        bp, hq = divmod(c, 4)
        bs, hs = 2 * bp, 4 * hq
        qrep = np.concatenate([qTf[bs:bs + NB, hs:hs + NH]] * 2, axis=2)
        # mex[t] = [128, 2*SUPW] for iteration t = ((h*NB+b)*NQP+qp)*NKT2+ktp
        # mex[t][p, j*SUPW+u] = maskT[key=(2ktp+j)*128+p, query=qp*SUPW+u]
        #                       * expbT[h][key, query] * SCALE
        mex = np.empty((T_ITER, P, 2 * SUPW), np.float16)
        for h in range(NH):
            for b in range(NB):
                m = np.where(maskT[bs + b], expbT[hs + h], np.float16(0))
                # [NKT2, 2, P, NQP, SUPW] -> [NQP, NKT2, P, 2, SUPW]
                mt = (m.reshape(NKT2, 2, P, NQP, SUPW)
                      .transpose(3, 0, 2, 1, 4)
                      .reshape(NQP, NKT2, P, 2 * SUPW))
                pairi = h * NB + b
                t0 = pairi * NQP * NKT2
                mex[t0:t0 + NQP * NKT2] = mt.reshape(
                    NQP * NKT2, P, 2 * SUPW)
        in_maps.append({
            "qT": np.ascontiguousarray(qrep),
            "kT": np.ascontiguousarray(kPar[bs:bs + NB, hs:hs + NH]),
            "vA": np.ascontiguousarray(vTile[bs:bs + NB, hs:hs + NH]),
            "mex": mex,
        })
    return in_maps


def kernel(q, k, v, mask, attn_bias):
    if "nc" not in _CACHE:
        _CACHE["nc"] = build_bass()
    nc = _CACHE["nc"]
    in_maps = make_in_maps(
        np.asarray(q, np.float32), np.asarray(k, np.float32),
        np.asarray(v, np.float32), np.asarray(mask, bool),
        np.asarray(attn_bias, np.float32),
    )
    rr = run_bass_kernel_spmd(
        nc, in_maps, list(range(8)), trace=TRACE,
        tmpdir=_CACHE.get("tmpdir"),
    )
    _CACHE["last_result"] = rr

    out = np.empty((B, H, N, D), np.float32)
    for c in range(8):
        bp, hq = divmod(c, 4)
        bs, hs = 2 * bp, 4 * hq
        oU = np.asarray(rr.results[c]["outU"]).astype(np.float32)  # [NB,NH,65,N]
        o = oU[:, :, :D, :] / oU[:, :, D:D + 1, :]
        out[bs:bs + NB, hs:hs + NH] = o.transpose(0, 1, 3, 2)
    return out


# revision 19
# speedup vs baseline: 1.3638x; 1.0036x over previous
"""Sparse attention (B=4,H=16,N=2048,D=64) on 8 trn2 NeuronCores.

Sharding: core c = bp*4 + hq handles batches [2bp, 2bp+1] x heads [4hq..4hq+3].
Per (b,h), with host-precomputed mex = mask^T * exp(bias^T) / 16 streamed in:
  P^T = exp(K Q^T/8) * mex          (exp on ACT, mul on DVE)
  [O~ ; denom]^T = [V | 1]^T @ P^T  (accumulated fp32 in PSUM)
Device ships unnormalized [O~; denom] as fp16; host divides + transposes.

Flat software-pipelined loop over 128 key-tile-pair iterations per core:
S-matmuls for iter t run 2 iterations ahead of the O-matmuls (lag-2) so
the ACT engine (exp, the pacer at ~1.01us/1024-wide tile) never starves
behind O-matmuls that wait on the DVE multiply.  Adjacent key tiles'
S-matmuls target PE row groups 0-63/64-127 (q replicated in both
partition halves; k parity-packed so odd tiles live in rows 64-127).
Measured engine load per core: ACT ~258us (saturated pacer), PE ~250us,
DVE ~180us, DMA ~76MB at ~84% duty.
"""

import numpy as np
import ml_dtypes

import concourse.bass as bass
from concourse import bacc
import concourse.mybir as mybir
import concourse.tile as tile
from concourse.bass_utils import run_bass_kernel_spmd

dt = mybir.dt
AF = mybir.ActivationFunctionType

B, H, N, D = 4, 16, 2048, 64
NB = 2    # batches per core
NH = 4    # heads per core
P = 128
NKT = N // P          # 16 key tiles
NKT2 = NKT // 2       # 8 key-tile pairs
QW = 512              # matmul free-dim (one PSUM bank of fp32)
SUPW = 1024           # S tile width / ACT width (2 PSUM banks)
NQP = N // SUPW       # 2 query supertiles
NPAIR = NB * NH       # 8 (b,h) pairs per core
T_ITER = NPAIR * NQP * NKT2   # 128 pipeline iterations
LAG = 2
SCALE = np.float32(1.0 / 16.0)   # folded into mex; cancels in normalization
TRACE = False

_CACHE = {}


def build_bass():
    nc = bacc.Bacc()
    # q^T scaled, replicated into both partition halves: [NB,NH,128,N]
    qT = nc.declare_dram_parameter("qT", [NB, NH, 2 * D, N], dt.float16, isOutput=False)
    # k^T parity-packed: rows 0-63 even key tiles, rows 64-127 odd: [NB,NH,128,N/2]
    kT = nc.declare_dram_parameter("kT", [NB, NH, 2 * D, N // 2], dt.float16, isOutput=False)
    # [V | 1] pre-tiled: [NB,NH,128,NKT*(D+1)]
    vA = nc.declare_dram_parameter("vA", [NB, NH, P, NKT * (D + 1)], dt.float16, isOutput=False)
    # mask*exp(bias)*SCALE, tiled per iteration: [T_ITER, 128, 2*SUPW]
    mex = nc.declare_dram_parameter("mex", [T_ITER, P, 2 * SUPW], dt.float16, isOutput=False)
    # unnormalized [O~; denom]^T per pair: [NB,NH,D+1,N]
    outU = nc.declare_dram_parameter("outU", [NB, NH, D + 1, N], dt.float16, isOutput=True)

    def sched(t):
        pair, r = divmod(t, NQP * NKT2)
        qp, ktp = divmod(r, NKT2)
        h, b = divmod(pair, NB)
        return pair, b, h, qp, ktp

    with tile.TileContext(nc) as tc:
        with (
            tc.tile_pool(name="qk", bufs=2) as qkpool,
            tc.tile_pool(name="vp", bufs=2) as vpool,
            tc.tile_pool(name="mex", bufs=6) as mpool,
            tc.tile_pool(name="pt", bufs=6) as ppool,
            tc.tile_pool(name="out", bufs=2) as opool_sb,
            tc.tile_pool(name="spsum", bufs=3, space="PSUM") as spool,
            tc.tile_pool(name="opsum", bufs=1, space="PSUM") as opool,
        ):
            qkv = [None] * NPAIR   # (qsb, ksb, vsb) per pair
            pts = [None] * T_ITER  # pt tile + indices for the O-side
            osum = [None]          # current opsum tiles

            def load_pair(p):
                h, b = divmod(p, NB)
                qt_ = qkpool.tile([2 * D, N], dt.float16, tag="q")
                kt_ = qkpool.tile([2 * D, N // 2], dt.float16, tag="k")
                vt_ = vpool.tile([P, NKT * (D + 1)], dt.float16, tag="v")
                if p == 0:
                    # ramp: spread the critical q/k loads over idle engine
                    # queues so they aren't starved by the mex prefetch burst.
                    # q goes at the HEAD of the sync queue (before all mex),
                    # k alone on the scalar queue (safe: no ACTIVATE yet).
                    nc.sync.dma_start(qt_, qT[b, h])
                    nc.scalar.dma_start(kt_, kT[b, h])
                    nc.gpsimd.dma_start(vt_, vA[b, h])
                else:
                    nc.gpsimd.dma_start(qt_, qT[b, h])
                    nc.gpsimd.dma_start(kt_, kT[b, h])
                    nc.gpsimd.dma_start(vt_, vA[b, h])
                qkv[p] = (qt_, kt_, vt_)

            load_pair(0)
            for t in range(T_ITER + LAG):
                # ---------------- S side: S-matmuls, exp, mex multiply ----
                if t < T_ITER:
                    pair, b, h, qp, ktp = sched(t)
                    q0 = qp * SUPW
                    # prefetch next pair's q/k/v mid-way through this pair
                    if t % (NQP * NKT2) == NKT2 and pair + 1 < NPAIR:
                        load_pair(pair + 1)
                    qsb, ksb, vsb = qkv[pair]

                    mexsb = mpool.tile([P, 2 * SUPW], dt.float16, tag="mex")
                    nc.sync.dma_start(mexsb, mex[t])

                    pt = ppool.tile([P, 2 * SUPW], dt.float16, tag="pt")
                    ssups = []
                    for j in range(2):
                        ss = spool.tile([P, SUPW], dt.float32, tag="s",
                                        name=f"ssup{j}")
                        ssups.append(ss)
                    # adjacent key tiles (j=0,1) -> PE row groups 0/64
                    ti = ktp  # column block in parity-packed k
                    for j in range(2):
                        for qi in range(2):
                            rg = j * D
                            nc.tensor.matmul(
                                ssups[j][:, qi * QW:(qi + 1) * QW],
                                ksb[rg:rg + D, ti * P:(ti + 1) * P],
                                qsb[rg:rg + D,
                                    q0 + qi * QW:q0 + (qi + 1) * QW],
                                start=True, stop=True,
                                tile_position=(rg, 0),
                            )
                    for j in range(2):
                        nc.scalar.activation(
                            pt[:, j * SUPW:(j + 1) * SUPW], ssups[j], AF.Exp)
                    nc.vector.tensor_mul(pt, pt, mexsb)
                    pts[t] = (pt, pair, b, h, qp, ktp)

                # ---------------- O side (lag-2): accumulate V~ @ P^T -----
                to = t - LAG
                if to >= 0:
                    pt, pair, b, h, qp, ktp = pts[to]
                    pts[to] = None
                    vsb = qkv[pair][2]
                    if ktp == 0:
                        osum[0] = [
                            opool.tile([D + 1, QW], dt.float32,
                                       tag=f"o{qi}", name=f"opsum{qi}")
                            for qi in range(2)
                        ]
                    opsum = osum[0]
                    for j in range(2):
                        kt = 2 * ktp + j
                        for qi in range(2):
                            nc.tensor.matmul(
                                opsum[qi],
                                vsb[:, kt * (D + 1):(kt + 1) * (D + 1)],
                                pt[:, j * SUPW + qi * QW:
                                   j * SUPW + (qi + 1) * QW],
                                start=(kt == 0), stop=(kt == NKT - 1),
                            )
                    if ktp == NKT2 - 1:
                        # drain unnormalized accumulators to SBUF as fp16
                        q0 = qp * SUPW
                        osb = opool_sb.tile([D + 1, SUPW], dt.float16,
                                            tag="osb")
                        for qi in range(2):
                            nc.vector.tensor_copy(
                                osb[:, qi * QW:(qi + 1) * QW], opsum[qi])
                        nc.gpsimd.dma_start(
                            outU[b, h, :, q0:q0 + SUPW], osb)
    nc.finalize()
    return nc


def make_in_maps(q, k, v, mask, attn_bias):
    scale = np.float32(D ** -0.5)
    qTf = (q.transpose(0, 1, 3, 2) * scale).astype(np.float16)   # [B,H,D,N]
    kTf = k.transpose(0, 1, 3, 2).astype(np.float16)             # [B,H,D,N]
    # parity-packed k^T: [B,H,2,D,N/2] -> [B,H,2D,N/2]
    kPar = np.ascontiguousarray(
        kTf.reshape(B, H, D, NKT2, 2, P).transpose(0, 1, 4, 2, 3, 5)
        .reshape(B, H, 2 * D, N // 2))
    vA = np.concatenate(
        [v, np.ones((B, H, N, 1), np.float32)], axis=-1
    ).astype(np.float16)                                         # [B,H,N,D+1]
    # pre-tiled: [B,H,P,NKT*(D+1)]
    vTile = np.ascontiguousarray(
        vA.reshape(B, H, NKT, P, D + 1).transpose(0, 1, 3, 2, 4)
        .reshape(B, H, P, NKT * (D + 1)))
    # transposed [key, query] views
    maskT = mask[:, 0].transpose(0, 2, 1)                        # [B,N,N] bool
    expbT = (np.exp(attn_bias[0].transpose(0, 2, 1),
                    dtype=np.float32) * SCALE).astype(np.float16)  # [H,N,N]

    in_maps = []
    for c in range(8):
        bp, hq = divmod(c, 4)
        bs, hs = 2 * bp, 4 * hq
        qrep = np.concatenate([qTf[bs:bs + NB, hs:hs + NH]] * 2, axis=2)
        # mex[t] = [128, 2*SUPW] for iteration t = ((h*NB+b)*NQP+qp)*NKT2+ktp
        # mex[t][p, j*SUPW+u] = maskT[key=(2ktp+j)*128+p, query=qp*SUPW+u]
        #                       * expbT[h][key, query] * SCALE
        mex = np.empty((T_ITER, P, 2 * SUPW), np.float16)
        for h in range(NH):
            for b in range(NB):
                m = np.where(maskT[bs + b], expbT[hs + h], np.float16(0))
                # [NKT2, 2, P, NQP, SUPW] -> [NQP, NKT2, P, 2, SUPW]
                mt = (m.reshape(NKT2, 2, P, NQP, SUPW)
                      .transpose(3, 0, 2, 1, 4)
                      .reshape(NQP, NKT2, P, 2 * SUPW))
                pairi = h * NB + b
                t0 = pairi * NQP * NKT2
                mex[t0:t0 + NQP * NKT2] = mt.reshape(
                    NQP * NKT2, P, 2 * SUPW)
        in_maps.append({
            "qT": np.ascontiguousarray(qrep),
            "kT": np.ascontiguousarray(kPar[bs:bs + NB, hs:hs + NH]),
            "vA": np.ascontiguousarray(vTile[bs:bs + NB, hs:hs + NH]),
            "mex": mex,
        })
    return in_maps


def kernel(q, k, v, mask, attn_bias):
    if "nc" not in _CACHE:
        _CACHE["nc"] = build_bass()
    nc = _CACHE["nc"]
    in_maps = make_in_maps(
        np.asarray(q, np.float32), np.asarray(k, np.float32),
        np.asarray(v, np.float32), np.asarray(mask, bool),
        np.asarray(attn_bias, np.float32),
    )
    rr = run_bass_kernel_spmd(
        nc, in_maps, list(range(8)), trace=TRACE,
        tmpdir=_CACHE.get("tmpdir"),
    )
    _CACHE["last_result"] = rr

    out = np.empty((B, H, N, D), np.float32)
    for c in range(8):
        bp, hq = divmod(c, 4)
        bs, hs = 2 * bp, 4 * hq
        oU = np.asarray(rr.results[c]["outU"]).astype(np.float32)  # [NB,NH,65,N]
        o = oU[:, :, :D, :] / oU[:, :, D:D + 1, :]
        out[bs:bs + NB, hs:hs + NH] = o.transpose(0, 1, 3, 2)
    return out
